# revision 5
# baseline (speedup 1.0000x reference)
"""CP-factorized embedding lookup on 8 TRN2 NeuronCores.

Reference computes full[a,b,c,d,e,f] = sum_r U0[a,r]*...*U5[f,r], reshapes to a
(50000, 512) table, and gathers rows by x. We never materialize the table:

  out[n, e] = sum_r (U0[a_n,r]*U1[b_n,r]*U2[c_n,r]) * (U3[d,r]*U4[e2,r]*U5[f,r])
            = sum_r V[n, r] * W[e, r]

with v = 1000a + 25b + c and e = 64d + 8e2 + f.

Per core (1024 indices, data-parallel over the 8192 total):
  1. decompose v -> (a, b+50, c+90) with exact f32 floor-division tricks
     (f32->i32 cast truncates on DVE; (v+0.5)*(1/d) has margin >= 4e-4)
  2. round-trip the 3 decomposed rows through a DRAM scratch, read back
     partition-broadcast -> rep[115, 1024]
  3. one-hot[115, 1024] = is_equal(rep, iota) ; gather via one PE matmul with
     block-diag stacked [U0;U1;U2] as lhsT -> psum[96, 1024] = three gathered
     factor rows; V = elementwise product of the three 32-row blocks
  4. W[32, 512] = Khatri-Rao of U3,U4,U5 built with two broadcast multiplies
  5. out chunk c: matmul(lhsT=V[:,128c:128c+128], rhs=W) -> psum[128, 512],
     masked copy (zero rows where v==0) -> SBUF -> DMA to DRAM
"""

import numpy as np

import concourse.bass as bass
import concourse.mybir as mybir
import concourse.tile as tile
from concourse import bacc
from concourse.bass_utils import run_bass_kernel_spmd

F32 = mybir.dt.float32
F32R = mybir.dt.float32r
I32 = mybir.dt.int32
ALU = mybir.AluOpType

N_CORES = 8
PER_CORE = 1024           # indices per core (8192 / 8)
NT = PER_CORE // 128      # 8 column-tiles of 128 indices
HALF = 512                # gather-matmul moving size (one PSUM bank)
EMB = 512
RANK = 32
KV = 115                  # 50 + 40 + 25 stacked vocab-factor rows
MV = 96                   # 3 * RANK stacked outputs

R1000 = float(np.float32(1.0 / 1000.0))
R25 = float(np.float32(1.0 / 25.0))

# f32 -> i32 conversion on HW DVE rounds to nearest even; CoreSim truncates.
# floor(v/d) is computed as cast((v + bias) * (1/d)) with bias chosen so the
# pre-cast value sits mid-interval for the respective rounding mode:
#   rint:  (v - (d/2 - 0.5))/d = floor + frac in [-0.4995, 0.4995]
#   trunc: (v + 0.5)/d         = floor + frac in [ 0.0005, 0.9995]
# Margins dwarf the ~1e-5 f32 multiply error, so the result is exact.
CAST_RINT = True


def _div_bias(d: float) -> float:
    return -(d / 2.0 - 0.5) if CAST_RINT else 0.5

# matmul operand dtype: float32r streams 1 row/cycle (vs 4 for float32) at
# ~2^-17 relative error, far inside tolerance for this problem.
MM_DT = F32


def build():
    nc = bacc.Bacc("TRN2", target_bir_lowering=False, debug=False)

    x = nc.dram_tensor("x", [PER_CORE], I32, kind="ExternalInput")
    U = [
        nc.dram_tensor(f"U{i}", [d, RANK], F32, kind="ExternalInput")
        for i, d in enumerate([50, 40, 25, 8, 8, 8])
    ]
    out = nc.dram_tensor("out", [PER_CORE, EMB], F32, kind="ExternalOutput")
    scratch = nc.dram_tensor("scratch", [3, NT, 128], F32)

    with tile.TileContext(nc) as tc:
        with (
            tc.tile_pool(name="const", bufs=1) as cpool,
            tc.tile_pool(name="work", bufs=2) as wpool,
            tc.tile_pool(name="vpsum", bufs=2, space="PSUM") as ppool,
            tc.tile_pool(name="osb", bufs=3) as opool,
            tc.tile_pool(name="opsum", bufs=4, space="PSUM") as oppool,
        ):
            # ---- x in column layout: xt[p, c] = x[c*128 + p], so chunk c of
            # the output (psum partition p <-> index 128c+p) lines up with
            # column c here (used for the padding mask).
            xt = cpool.tile([128, NT], I32)
            nc.sync.dma_start(out=xt[:], in_=x[:].rearrange("(f p) -> p f", p=128))
            vf = cpool.tile([128, NT], F32)
            nc.vector.tensor_copy(out=vf[:], in_=xt[:])
            mask = cpool.tile([128, NT], F32)
            nc.vector.tensor_scalar(
                out=mask[:], in0=vf[:], scalar1=0.0, scalar2=None, op0=ALU.not_equal
            )

            # ---- decomposition (exact: see module docstring)
            a_i = cpool.tile([128, NT], I32)
            nc.vector.tensor_scalar(
                out=a_i[:], in0=vf[:], scalar1=_div_bias(1000.0), scalar2=R1000,
                op0=ALU.add, op1=ALU.mult,
            )
            q_i = cpool.tile([128, NT], I32)
            nc.vector.tensor_scalar(
                out=q_i[:], in0=vf[:], scalar1=_div_bias(25.0), scalar2=R25,
                op0=ALU.add, op1=ALU.mult,
            )
            abc = cpool.tile([128, 3 * NT], F32)
            af = abc[:, 0:NT]
            nc.vector.tensor_copy(out=af, in_=a_i[:])
            qf = cpool.tile([128, NT], F32)
            nc.vector.tensor_copy(out=qf[:], in_=q_i[:])
            t1 = cpool.tile([128, NT], F32)
            nc.vector.tensor_scalar(  # 40a - 50
                out=t1[:], in0=af, scalar1=40.0, scalar2=50.0,
                op0=ALU.mult, op1=ALU.subtract,
            )
            nc.vector.tensor_tensor(  # b + 50 = q25 - (40a - 50)
                out=abc[:, NT:2 * NT], in0=qf[:], in1=t1[:], op=ALU.subtract
            )
            t2 = cpool.tile([128, NT], F32)
            nc.vector.tensor_scalar(  # 25q - 90
                out=t2[:], in0=qf[:], scalar1=25.0, scalar2=90.0,
                op0=ALU.mult, op1=ALU.subtract,
            )
            nc.vector.tensor_tensor(  # c + 90 = v - (25q - 90)
                out=abc[:, 2 * NT:3 * NT], in0=vf[:], in1=t2[:], op=ALU.subtract
            )

            # ---- scratch round trip to transpose (a,b50,c90) into row layout
            nc.sync.dma_start(
                out=scratch[:].rearrange("v f p -> p v f"),
                in_=abc[:].rearrange("p (v f) -> p v f", v=3),
            )
            rep = cpool.tile([KV, PER_CORE], F32)
            for var, (lo, nrow) in enumerate(((0, 50), (50, 40), (90, 25))):
                nc.sync.dma_start(
                    out=rep[lo:lo + nrow, :],
                    in_=scratch[var]
                    .rearrange("f p -> (f p)")
                    .unsqueeze(0)
                    .partition_broadcast(nrow),
                )

            # ---- one-hot
            iota_i = cpool.tile([KV, 1], I32)
            nc.gpsimd.iota(iota_i[:], pattern=[[0, 1]], base=0, channel_multiplier=1)
            iota_f = cpool.tile([KV, 1], F32)
            nc.gpsimd.tensor_copy(out=iota_f[:], in_=iota_i[:])
            onehot = cpool.tile([KV, PER_CORE], F32)
            nc.gpsimd.tensor_scalar(
                out=onehot[:], in0=rep[:], scalar1=iota_f[:], scalar2=None,
                op0=ALU.is_equal,
            )

            # ---- stacked factors (block diagonal) for the gather matmul
            ublk = cpool.tile([KV, MV], F32)
            nc.vector.memset(ublk[:], 0.0)
            nc.sync.dma_start(out=ublk[0:50, 0:32], in_=U[0][:])
            nc.sync.dma_start(out=ublk[50:90, 32:64], in_=U[1][:])
            nc.sync.dma_start(out=ublk[90:115, 64:96], in_=U[2][:])

            # ---- W[r, e] = U3[d,r] * U4[e2,r] * U5[f,r],  e = 64d + 8e2 + f
            u3t = cpool.tile([RANK, 8], F32)
            u4t = cpool.tile([RANK, 8], F32)
            u5t = cpool.tile([RANK, 8], F32)
            nc.sync.dma_start(out=u3t[:], in_=U[3][:].rearrange("d r -> r d"))
            nc.sync.dma_start(out=u4t[:], in_=U[4][:].rearrange("d r -> r d"))
            nc.sync.dma_start(out=u5t[:], in_=U[5][:].rearrange("d r -> r d"))
            t45 = cpool.tile([RANK, 64], F32)
            nc.vector.tensor_tensor(
                out=t45[:].rearrange("r (e f) -> r e f", e=8),
                in0=u4t[:].unsqueeze(2).broadcast_to([RANK, 8, 8]),
                in1=u5t[:].unsqueeze(1).broadcast_to([RANK, 8, 8]),
                op=ALU.mult,
            )
            wt = cpool.tile([RANK, EMB], F32)
            nc.vector.tensor_tensor(
                out=wt[:].rearrange("r (d ef) -> r d ef", d=8),
                in0=u3t[:].unsqueeze(2).broadcast_to([RANK, 8, 64]),
                in1=t45[:].unsqueeze(1).broadcast_to([RANK, 8, 64]),
                op=ALU.mult,
            )

            # ---- gather + 3-way product: V[r, n]
            vt = cpool.tile([RANK, PER_CORE], F32)
            for h in range(PER_CORE // HALF):
                pv = ppool.tile([MV, HALF], F32)
                nc.tensor.matmul(
                    pv[:],
                    lhsT=ublk[:].bitcast(MM_DT),
                    rhs=onehot[:, h * HALF:(h + 1) * HALF].bitcast(MM_DT),
                    start=True, stop=True,
                )
                # DVE may read only one PSUM operand per op: stage block 0
                # to SBUF on the Scalar engine first.
                s0 = wpool.tile([RANK, HALF], F32)
                nc.scalar.copy(out=s0[:], in_=pv[0:32, :])
                v01 = wpool.tile([RANK, HALF], F32)
                nc.vector.tensor_tensor(
                    out=v01[:], in0=s0[:], in1=pv[32:64, :], op=ALU.mult
                )
                nc.vector.tensor_tensor(
                    out=vt[:, h * HALF:(h + 1) * HALF],
                    in0=v01[:], in1=pv[64:96, :], op=ALU.mult,
                )

            # ---- output chunks
            for c in range(NT):
                po = oppool.tile([128, EMB], F32)
                nc.tensor.matmul(
                    po[:],
                    lhsT=vt[:, c * 128:(c + 1) * 128].bitcast(MM_DT),
                    rhs=wt[:].bitcast(MM_DT),
                    start=True, stop=True,
                )
                osb = opool.tile([128, EMB], F32)
                nc.scalar.activation(
                    out=osb[:], in_=po[:],
                    func=mybir.ActivationFunctionType.Copy,
                    scale=mask[:, c:c + 1],
                )
                nc.sync.dma_start(out=out[c * 128:(c + 1) * 128, :], in_=osb[:])

    nc.compile()
    return nc


_CACHE: dict = {}


def _get_nc():
    if "nc" not in _CACHE:
        _CACHE["nc"] = build()
    return _CACHE["nc"]


def run(inputs, **spmd_kwargs):
    nc = _get_nc()
    x = np.ascontiguousarray(inputs["x"].reshape(-1), dtype=np.int32)
    us = [
        np.ascontiguousarray(inputs[f"U{j}"], dtype=np.float32) for j in range(6)
    ]
    in_maps = []
    for i in range(N_CORES):
        m = {"x": x[i * PER_CORE:(i + 1) * PER_CORE]}
        for j in range(6):
            m[f"U{j}"] = us[j]
        in_maps.append(m)
    res = run_bass_kernel_spmd(
        nc, in_maps, core_ids=list(range(N_CORES)), **spmd_kwargs
    )
    shards = [np.asarray(res.results[i]["out"]) for i in range(N_CORES)]
    full = np.concatenate(shards, axis=0).reshape(4, 2048, EMB)
    return full.astype(np.float32, copy=False), res


def kernel(**inputs) -> np.ndarray:
    return run(inputs)[0]


# revision 13
# speedup vs baseline: 2.0265x; 2.0265x over previous
"""CP-factorized embedding lookup on 8 TRN2 NeuronCores.

Reference computes full[a,b,c,d,e,f] = sum_r U0[a,r]*...*U5[f,r], reshapes to a
(50000, 512) table, and gathers rows by x. We never materialize the table:

  out[n, e] = sum_r (U0[a_n,r]*U1[b_n,r]*U2[c_n,r]) * (U3[d,r]*U4[e2,r]*U5[f,r])
            = sum_r V[n, r] * W[e, r]

with v = 1000a + 25b + c and e = 64d + 8e2 + f.

Per core (1024 indices, data-parallel over the 8192 total):
  1. load x as [8, 128] (contiguous, n = f*128 + p), decompose v -> (a, b+50,
     c+90) there with exact f32 floor-division tricks (HW f32->i32 cast is
     round-to-nearest-even; (v - (d/2 - 0.5))*(1/d) rounds to floor(v/d))
  2. round-trip the 3 decomposed rows through a DRAM scratch, read back
     partition-broadcast -> rep[115, 1024]
  3. one-hot[115, 1024] = is_equal(rep, iota) ; gather via one PE matmul with
     block-diag stacked [U0;U1;U2] as lhsT -> psum[96, 1024] = three gathered
     factor rows; V = elementwise product of the three 32-row blocks
  4. W[32, 512] = Khatri-Rao of U3,U4,U5 built with two broadcast multiplies
     (U3/U4/U5 transposed on-chip through the PE)
  5. out chunk c: matmul(lhsT=V[:,128c:128c+128], rhs=W) -> psum[128, 512],
     masked copy (zero rows where v==0; mask transposed to column layout
     through the PE) -> SBUF -> DMA to DRAM

Matmul operands are produced as float32r (tf32-like, 1 row/cycle vs 4 for
float32); the one-hot entries are exact in any dtype and the factor rounding
error is ~1e-5 relative, far inside tolerance.
"""

import numpy as np

import concourse.bass as bass
import concourse.mybir as mybir
import concourse.tile as tile
from concourse import bacc
from concourse.bass_utils import run_bass_kernel_spmd
from concourse.masks import make_identity

F32 = mybir.dt.float32
F32R = mybir.dt.float32r
I32 = mybir.dt.int32
ALU = mybir.AluOpType

N_CORES = 8
PER_CORE = 1024           # indices per core (8192 / 8)
NT = PER_CORE // 128      # 8 column-tiles of 128 indices
HALF = 512                # gather-matmul moving size (one PSUM bank)
EMB = 512
RANK = 32
KV = 115                  # 50 + 40 + 25 stacked vocab-factor rows
MV = 96                   # 3 * RANK stacked outputs

R1000 = float(np.float32(1.0 / 1000.0))
R25 = float(np.float32(1.0 / 25.0))

# f32 -> i32 conversion on HW DVE rounds to nearest even; CoreSim truncates.
# floor(v/d) is computed as cast((v + bias) * (1/d)) with bias chosen so the
# pre-cast value sits mid-interval for the respective rounding mode:
#   rint:  (v - (d/2 - 0.5))/d = floor + frac in [-0.4995, 0.4995]
#   trunc: (v + 0.5)/d         = floor + frac in [ 0.0005, 0.9995]
# Margins dwarf the ~1e-5 f32 multiply error, so the result is exact.
CAST_RINT = True

# matmul operand dtype: float32r streams 1 row/cycle (vs 4 for float32).
MM_DT = F32R


def _div_bias(d: float) -> float:
    return -(d / 2.0 - 0.5) if CAST_RINT else 0.5


def build():
    nc = bacc.Bacc("TRN2", target_bir_lowering=False, debug=False)

    x = nc.dram_tensor("x", [PER_CORE], I32, kind="ExternalInput")
    U = [
        nc.dram_tensor(f"U{i}", [d, RANK], F32, kind="ExternalInput")
        for i, d in enumerate([50, 40, 25, 8, 8, 8])
    ]
    out = nc.dram_tensor("out", [PER_CORE, EMB], F32, kind="ExternalOutput")
    scratch = nc.dram_tensor("scratch", [3, NT, 128], F32)

    with tile.TileContext(nc) as tc:
        with (
            tc.tile_pool(name="const", bufs=1) as cpool,
            tc.tile_pool(name="work", bufs=2) as wpool,
            tc.tile_pool(name="vpsum", bufs=2, space="PSUM") as ppool,
            tc.tile_pool(name="tpsum", bufs=1, space="PSUM") as tpool,
            tc.tile_pool(name="osb", bufs=3) as opool,
            tc.tile_pool(name="opsum", bufs=4, space="PSUM") as oppool,
        ):
            ident = cpool.tile([24, 24], F32)
            make_identity(nc, ident[:])

            # ---- x as [8, 128]: xt[f, p] = x[f*128 + p] (contiguous rows)
            xt = cpool.tile([8, 128], I32)
            nc.sync.dma_start(out=xt[:], in_=x[:].rearrange("(f p) -> f p", p=128))
            vf = cpool.tile([8, 128], F32)
            nc.vector.tensor_copy(out=vf[:], in_=xt[:])

            # mask in column layout [128, NT] via PE transpose of [8, 128]
            mask8 = cpool.tile([8, 128], F32)
            nc.vector.tensor_scalar(
                out=mask8[:], in0=vf[:], scalar1=0.0, scalar2=None,
                op0=ALU.not_equal,
            )
            mask_ps = tpool.tile([128, 8], F32, tag="tps")
            nc.tensor.transpose(mask_ps[:], mask8[:], ident[0:8, 0:8])
            mask = cpool.tile([128, NT], F32)
            nc.scalar.copy(out=mask[:], in_=mask_ps[:])

            # ---- decomposition (exact: see module docstring)
            a_i = cpool.tile([8, 128], I32)
            nc.vector.tensor_scalar(
                out=a_i[:], in0=vf[:], scalar1=_div_bias(1000.0), scalar2=R1000,
                op0=ALU.add, op1=ALU.mult,
            )
            q_i = cpool.tile([8, 128], I32)
            nc.vector.tensor_scalar(
                out=q_i[:], in0=vf[:], scalar1=_div_bias(25.0), scalar2=R25,
                op0=ALU.add, op1=ALU.mult,
            )
            # abc[f, var*128 + p]: var-major so the scratch DMA writes 512B runs
            abc = cpool.tile([8, 3 * 128], F32)
            af = abc[:, 0:128]
            nc.vector.tensor_copy(out=af, in_=a_i[:])
            qf = cpool.tile([8, 128], F32)
            nc.vector.tensor_copy(out=qf[:], in_=q_i[:])
            t1 = cpool.tile([8, 128], F32)
            nc.vector.tensor_scalar(  # 40a - 50
                out=t1[:], in0=af, scalar1=40.0, scalar2=50.0,
                op0=ALU.mult, op1=ALU.subtract,
            )
            nc.vector.tensor_tensor(  # b + 50 = q25 - (40a - 50)
                out=abc[:, 128:256], in0=qf[:], in1=t1[:], op=ALU.subtract
            )
            t2 = cpool.tile([8, 128], F32)
            nc.vector.tensor_scalar(  # 25q - 90
                out=t2[:], in0=qf[:], scalar1=25.0, scalar2=90.0,
                op0=ALU.mult, op1=ALU.subtract,
            )
            nc.vector.tensor_tensor(  # c + 90 = v - (25q - 90)
                out=abc[:, 256:384], in0=vf[:], in1=t2[:], op=ALU.subtract
            )

            # ---- scratch round trip to transpose (a,b50,c90) into row layout
            nc.sync.dma_start(
                out=scratch[:].rearrange("v f p -> f v p"),
                in_=abc[:].rearrange("f (v p) -> f v p", v=3),
            )
            rep = cpool.tile([KV, PER_CORE], F32)
            for var, (lo, nrow) in enumerate(((0, 50), (50, 40), (90, 25))):
                nc.sync.dma_start(
                    out=rep[lo:lo + nrow, :],
                    in_=scratch[var]
                    .rearrange("f p -> (f p)")
                    .unsqueeze(0)
                    .partition_broadcast(nrow),
                )

            # ---- one-hot
            iota_i = cpool.tile([KV, 1], I32)
            nc.gpsimd.iota(iota_i[:], pattern=[[0, 1]], base=0, channel_multiplier=1)
            iota_f = cpool.tile([KV, 1], F32)
            nc.gpsimd.tensor_copy(out=iota_f[:], in_=iota_i[:])
            onehot = cpool.tile([KV, PER_CORE], MM_DT)
            nc.vector.tensor_scalar(
                out=onehot[:], in0=rep[:], scalar1=iota_f[:], scalar2=None,
                op0=ALU.is_equal,
            )

            # ---- stacked factors (block diagonal) for the gather matmul.
            # Staged in f32 (memset lacks an f32r encoding), then one tiny
            # DVE copy produces the f32r-rounded matmul operand.
            ublk_f = cpool.tile([KV, MV], F32)
            nc.vector.memset(ublk_f[:], 0.0)
            nc.sync.dma_start(out=ublk_f[0:50, 0:32], in_=U[0][:])
            nc.sync.dma_start(out=ublk_f[50:90, 32:64], in_=U[1][:])
            nc.sync.dma_start(out=ublk_f[90:115, 64:96], in_=U[2][:])
            ublk = cpool.tile([KV, MV], MM_DT)
            nc.vector.tensor_copy(out=ublk[:], in_=ublk_f[:])

            # ---- W[r, e] = U3[d,r] * U4[e2,r] * U5[f,r],  e = 64d + 8e2 + f
            # U3/U4/U5 transposed on-chip: [24, 32] -> PE -> [32, 24]
            u345 = cpool.tile([24, RANK], F32)
            nc.sync.dma_start(out=u345[0:8, :], in_=U[3][:])
            nc.sync.dma_start(out=u345[8:16, :], in_=U[4][:])
            nc.sync.dma_start(out=u345[16:24, :], in_=U[5][:])
            u345t_ps = tpool.tile([RANK, 24], F32, tag="tps")
            nc.tensor.transpose(u345t_ps[:], u345[:], ident[:])
            u345t = cpool.tile([RANK, 24], F32)
            nc.scalar.copy(out=u345t[:], in_=u345t_ps[:])
            t45 = cpool.tile([RANK, 64], F32)
            nc.vector.tensor_tensor(
                out=t45[:].rearrange("r (e f) -> r e f", e=8),
                in0=u345t[:, 8:16].unsqueeze(2).broadcast_to([RANK, 8, 8]),
                in1=u345t[:, 16:24].unsqueeze(1).broadcast_to([RANK, 8, 8]),
                op=ALU.mult,
            )
            wt = cpool.tile([RANK, EMB], MM_DT)
            nc.vector.tensor_tensor(
                out=wt[:].rearrange("r (d ef) -> r d ef", d=8),
                in0=u345t[:, 0:8].unsqueeze(2).broadcast_to([RANK, 8, 64]),
                in1=t45[:].unsqueeze(1).broadcast_to([RANK, 8, 64]),
                op=ALU.mult,
            )

            # ---- gather + 3-way product: V[r, n]
            vt = cpool.tile([RANK, PER_CORE], MM_DT)
            for h in range(PER_CORE // HALF):
                pv = ppool.tile([MV, HALF], F32)
                nc.tensor.matmul(
                    pv[:],
                    lhsT=ublk[:],
                    rhs=onehot[:, h * HALF:(h + 1) * HALF],
                    start=True, stop=True,
                )
                # DVE may read only one PSUM operand per op: stage block 0
                # to SBUF on the Scalar engine first.
                s0 = wpool.tile([RANK, HALF], F32)
                nc.scalar.copy(out=s0[:], in_=pv[0:32, :])
                v01 = wpool.tile([RANK, HALF], F32)
                nc.vector.tensor_tensor(
                    out=v01[:], in0=s0[:], in1=pv[32:64, :], op=ALU.mult
                )
                nc.vector.tensor_tensor(
                    out=vt[:, h * HALF:(h + 1) * HALF],
                    in0=v01[:], in1=pv[64:96, :], op=ALU.mult,
                )

            # ---- output chunks
            for c in range(NT):
                po = oppool.tile([128, EMB], F32)
                nc.tensor.matmul(
                    po[:],
                    lhsT=vt[:, c * 128:(c + 1) * 128],
                    rhs=wt[:],
                    start=True, stop=True,
                )
                osb = opool.tile([128, EMB], F32)
                nc.scalar.activation(
                    out=osb[:], in_=po[:],
                    func=mybir.ActivationFunctionType.Copy,
                    scale=mask[:, c:c + 1],
                )
                nc.sync.dma_start(out=out[c * 128:(c + 1) * 128, :], in_=osb[:])

    nc.compile()
    return nc


_CACHE: dict = {}


def _get_nc():
    if "nc" not in _CACHE:
        _CACHE["nc"] = build()
    return _CACHE["nc"]


def run(inputs, **spmd_kwargs):
    nc = _get_nc()
    x = np.ascontiguousarray(inputs["x"].reshape(-1), dtype=np.int32)
    us = [
        np.ascontiguousarray(inputs[f"U{j}"], dtype=np.float32) for j in range(6)
    ]
    in_maps = []
    for i in range(N_CORES):
        m = {"x": x[i * PER_CORE:(i + 1) * PER_CORE]}
        for j in range(6):
            m[f"U{j}"] = us[j]
        in_maps.append(m)
    res = run_bass_kernel_spmd(
        nc, in_maps, core_ids=list(range(N_CORES)), **spmd_kwargs
    )
    shards = [np.asarray(res.results[i]["out"]) for i in range(N_CORES)]
    full = np.concatenate(shards, axis=0).reshape(4, 2048, EMB)
    return full.astype(np.float32, copy=False), res


def kernel(**inputs) -> np.ndarray:
    return run(inputs)[0]


# revision 16
# speedup vs baseline: 2.1442x; 1.0581x over previous
"""CP-factorized embedding lookup on 8 TRN2 NeuronCores.

Reference computes full[a,b,c,d,e,f] = sum_r U0[a,r]*...*U5[f,r], reshapes to a
(50000, 512) table, and gathers rows by x. We never materialize the table:

  out[n, e] = sum_r (U0[a_n,r]*U1[b_n,r]*U2[c_n,r]) * (U3[d,r]*U4[e2,r]*U5[f,r])
            = sum_r V[n, r] * W[e, r]

with v = 1000a + 25b + c and e = 64d + 8e2 + f.

Per core (1024 indices, data-parallel over the 8192 total):
  1. broadcast x across 115 partitions (50+40+25 stacked factor rows) and
     decompose it in place with per-partition constants:
       rows  0:50  -> a      = floor(v/1000)
       rows 50:90  -> b + 50 = floor(v/25) - 40*floor(v/1000) + 50
       rows 90:115 -> c + 90 = v - 25*floor(v/25) + 90
     floor(v/d) = f32->i32 cast of (v + bias)*(1/d); the HW cast rounds to
     nearest even, bias = -(d/2 - 0.5) puts the value mid-interval, so the
     result is exact. One uniform 7-op DVE chain covers all three row blocks
     (constants differ per partition, supplied as a small input table).
  2. one-hot[115, 1024] = is_equal(decomposed, iota); gather via one PE
     matmul with block-diag stacked [U0;U1;U2] as lhsT -> psum[96, 1024];
     V = elementwise product of the three 32-row blocks
  3. W[32, 512] = Khatri-Rao of U3,U4,U5 built with two broadcast multiplies
     (U3/U4/U5 transposed on-chip through the PE)
  4. out chunk c: matmul(lhsT=V[:,128c:128c+128], rhs=W) -> psum[128, 512],
     masked copy (zero rows where v==0; mask transposed to column layout
     through the PE) -> SBUF -> DMA to DRAM

Matmul operands are produced as float32r (tf32-like, 1 row/cycle vs 4 for
float32); one-hot entries are exact in any dtype and the factor rounding
error is ~1e-4 relative, far inside tolerance.
"""

import numpy as np

import concourse.bass as bass
import concourse.mybir as mybir
import concourse.tile as tile
from concourse import bacc
from concourse.bass_utils import run_bass_kernel_spmd

F32 = mybir.dt.float32
F32R = mybir.dt.float32r
I32 = mybir.dt.int32
ALU = mybir.AluOpType

N_CORES = 8
PER_CORE = 1024           # indices per core (8192 / 8)
NT = PER_CORE // 128      # 8 column-tiles of 128 indices
HALF = 512                # gather-matmul moving size (one PSUM bank)
EMB = 512
RANK = 32
KV = 115                  # 50 + 40 + 25 stacked vocab-factor rows
MV = 96                   # 3 * RANK stacked outputs

R1000 = float(np.float32(1.0 / 1000.0))
R25 = float(np.float32(1.0 / 25.0))
R40 = float(np.float32(1.0 / 40.0))

# matmul operand dtype: float32r streams 1 row/cycle (vs 4 for float32).
MM_DT = F32R


def _const_table() -> np.ndarray:
    """[115, 7] per-partition constants: b1, R1, b2, R2, K, OFF, iota.

    Row block decomposition chain (s1, s2 are f32->i32->f32 floor stages):
      s1 = floor((v + b1) * R1);  s2 = floor((s1 + b2) * R2)
      value = s1 - (K*s2 - OFF)  ; onehot = (value == iota)
    """
    cc = np.zeros((KV, 7), np.float32)
    rows = ((0, 50), (50, 90), (90, 115))
    vals = [
        # a = floor(v/1000); s2 unused (K=0)
        (-499.5, R1000, 0.0, 1.0, 0.0, 0.0),
        # s1 = q25 = floor(v/25); s2 = floor(q25/40) = a; b+50 = q25 - 40a + 50
        (-12.0, R25, -19.5, R40, 40.0, 50.0),
        # s1 = v; s2 = q25; c+90 = v - 25*q25 + 90
        (0.0, 1.0, -12.0, R25, 25.0, 90.0),
    ]
    for (lo, hi), v6 in zip(rows, vals):
        cc[lo:hi, 0:6] = np.float32(v6)
    cc[:, 6] = np.arange(KV, dtype=np.float32)
    return cc


def build():
    nc = bacc.Bacc("TRN2", target_bir_lowering=False, debug=False)

    x = nc.dram_tensor("x", [PER_CORE], I32, kind="ExternalInput")
    U = [
        nc.dram_tensor(f"U{i}", [d, RANK], F32, kind="ExternalInput")
        for i, d in enumerate([50, 40, 25, 8, 8, 8])
    ]
    cc_d = nc.dram_tensor("cc", [KV, 7], F32, kind="ExternalInput")
    idm_d = nc.dram_tensor("idm", [24, 24], F32, kind="ExternalInput")
    out = nc.dram_tensor("out", [PER_CORE, EMB], F32, kind="ExternalOutput")

    with tile.TileContext(nc) as tc:
        with (
            tc.tile_pool(name="const", bufs=1) as cpool,
            tc.tile_pool(name="work", bufs=2) as wpool,
            tc.tile_pool(name="vpsum", bufs=2, space="PSUM") as ppool,
            tc.tile_pool(name="tpsum", bufs=1, space="PSUM") as tpool,
            tc.tile_pool(name="osb", bufs=3) as opool,
            tc.tile_pool(name="opsum", bufs=4, space="PSUM") as oppool,
        ):
            # ---- broadcast x across the 115 stacked factor rows
            xrep = cpool.tile([KV, PER_CORE], I32)
            nc.sync.dma_start(
                out=xrep[:], in_=x[:].unsqueeze(0).partition_broadcast(KV)
            )
            cc = cpool.tile([KV, 7], F32)
            nc.sync.dma_start(out=cc[:], in_=cc_d[:])
            idm = cpool.tile([24, 24], F32)
            nc.sync.dma_start(out=idm[:], in_=idm_d[:])

            # ---- padding mask: x as [8, 128] -> (v != 0) -> PE transpose
            # into column layout (mask[p, c] belongs to index n = c*128 + p,
            # matching output chunk c's psum partition p)
            xt = cpool.tile([8, 128], I32)
            nc.sync.dma_start(out=xt[:], in_=x[:].rearrange("(f p) -> f p", p=128))
            mask8 = cpool.tile([8, 128], F32)
            nc.vector.tensor_scalar(
                out=mask8[:], in0=xt[:], scalar1=0.0, scalar2=None,
                op0=ALU.not_equal,
            )
            mask_ps = tpool.tile([128, 8], F32, tag="tps")
            nc.tensor.transpose(mask_ps[:], mask8[:], idm[0:8, 0:8])
            mask = cpool.tile([128, NT], F32)
            nc.scalar.copy(out=mask[:], in_=mask_ps[:])

            # ---- decomposition chain (see _const_table)
            s1_i = cpool.tile([KV, PER_CORE], I32)
            nc.vector.tensor_scalar(
                out=s1_i[:], in0=xrep[:], scalar1=cc[:, 0:1], scalar2=cc[:, 1:2],
                op0=ALU.add, op1=ALU.mult,
            )
            s2_i = cpool.tile([KV, PER_CORE], I32)
            nc.vector.tensor_scalar(
                out=s2_i[:], in0=s1_i[:], scalar1=cc[:, 2:3], scalar2=cc[:, 3:4],
                op0=ALU.add, op1=ALU.mult,
            )
            tk = cpool.tile([KV, PER_CORE], F32)
            nc.vector.tensor_scalar(
                out=tk[:], in0=s2_i[:], scalar1=cc[:, 4:5], scalar2=cc[:, 5:6],
                op0=ALU.mult, op1=ALU.subtract,
            )
            diff = cpool.tile([KV, PER_CORE], F32)
            nc.vector.tensor_tensor(
                out=diff[:], in0=s1_i[:], in1=tk[:], op=ALU.subtract
            )
            onehot = cpool.tile([KV, PER_CORE], MM_DT)
            nc.vector.tensor_scalar(
                out=onehot[:], in0=diff[:], scalar1=cc[:, 6:7], scalar2=None,
                op0=ALU.is_equal,
            )

            # ---- stacked factors (block diagonal) for the gather matmul.
            # Staged in f32 (memset lacks an f32r encoding), then one tiny
            # DVE copy produces the f32r-rounded matmul operand.
            ublk_f = cpool.tile([KV, MV], F32)
            nc.vector.memset(ublk_f[:], 0.0)
            nc.sync.dma_start(out=ublk_f[0:50, 0:32], in_=U[0][:])
            nc.sync.dma_start(out=ublk_f[50:90, 32:64], in_=U[1][:])
            nc.sync.dma_start(out=ublk_f[90:115, 64:96], in_=U[2][:])
            ublk = cpool.tile([KV, MV], MM_DT)
            nc.vector.tensor_copy(out=ublk[:], in_=ublk_f[:])

            # ---- W[r, e] = U3[d,r] * U4[e2,r] * U5[f,r],  e = 64d + 8e2 + f
            # U3/U4/U5 transposed on-chip: [24, 32] -> PE -> [32, 24]
            u345 = cpool.tile([24, RANK], F32)
            nc.sync.dma_start(out=u345[0:8, :], in_=U[3][:])
            nc.sync.dma_start(out=u345[8:16, :], in_=U[4][:])
            nc.sync.dma_start(out=u345[16:24, :], in_=U[5][:])
            u345t_ps = tpool.tile([RANK, 24], F32, tag="tps")
            nc.tensor.transpose(u345t_ps[:], u345[:], idm[:])
            u345t = cpool.tile([RANK, 24], F32)
            nc.scalar.copy(out=u345t[:], in_=u345t_ps[:])
            t45 = cpool.tile([RANK, 64], F32)
            nc.vector.tensor_tensor(
                out=t45[:].rearrange("r (e f) -> r e f", e=8),
                in0=u345t[:, 8:16].unsqueeze(2).broadcast_to([RANK, 8, 8]),
                in1=u345t[:, 16:24].unsqueeze(1).broadcast_to([RANK, 8, 8]),
                op=ALU.mult,
            )
            wt = cpool.tile([RANK, EMB], MM_DT)
            nc.vector.tensor_tensor(
                out=wt[:].rearrange("r (d ef) -> r d ef", d=8),
                in0=u345t[:, 0:8].unsqueeze(2).broadcast_to([RANK, 8, 64]),
                in1=t45[:].unsqueeze(1).broadcast_to([RANK, 8, 64]),
                op=ALU.mult,
            )

            # ---- gather + 3-way product: V[r, n]
            vt = cpool.tile([RANK, PER_CORE], MM_DT)
            for h in range(PER_CORE // HALF):
                pv = ppool.tile([MV, HALF], F32)
                nc.tensor.matmul(
                    pv[:],
                    lhsT=ublk[:],
                    rhs=onehot[:, h * HALF:(h + 1) * HALF],
                    start=True, stop=True,
                )
                # DVE may read only one PSUM operand per op: stage block 0
                # to SBUF on the Scalar engine first.
                s0 = wpool.tile([RANK, HALF], F32)
                nc.scalar.copy(out=s0[:], in_=pv[0:32, :])
                v01 = wpool.tile([RANK, HALF], F32)
                nc.vector.tensor_tensor(
                    out=v01[:], in0=s0[:], in1=pv[32:64, :], op=ALU.mult
                )
                nc.vector.tensor_tensor(
                    out=vt[:, h * HALF:(h + 1) * HALF],
                    in0=v01[:], in1=pv[64:96, :], op=ALU.mult,
                )

            # ---- output chunks
            for c in range(NT):
                po = oppool.tile([128, EMB], F32)
                nc.tensor.matmul(
                    po[:],
                    lhsT=vt[:, c * 128:(c + 1) * 128],
                    rhs=wt[:],
                    start=True, stop=True,
                )
                osb = opool.tile([128, EMB], F32)
                nc.scalar.activation(
                    out=osb[:], in_=po[:],
                    func=mybir.ActivationFunctionType.Copy,
                    scale=mask[:, c:c + 1],
                )
                nc.sync.dma_start(out=out[c * 128:(c + 1) * 128, :], in_=osb[:])

    nc.compile()
    return nc


_CACHE: dict = {}


def _get_nc():
    if "nc" not in _CACHE:
        _CACHE["nc"] = build()
    return _CACHE["nc"]


def run(inputs, **spmd_kwargs):
    nc = _get_nc()
    x = np.ascontiguousarray(inputs["x"].reshape(-1), dtype=np.int32)
    us = [
        np.ascontiguousarray(inputs[f"U{j}"], dtype=np.float32) for j in range(6)
    ]
    cc = _const_table()
    idm = np.eye(24, dtype=np.float32)
    in_maps = []
    for i in range(N_CORES):
        m = {"x": x[i * PER_CORE:(i + 1) * PER_CORE], "cc": cc, "idm": idm}
        for j in range(6):
            m[f"U{j}"] = us[j]
        in_maps.append(m)
    res = run_bass_kernel_spmd(
        nc, in_maps, core_ids=list(range(N_CORES)), **spmd_kwargs
    )
    shards = [np.asarray(res.results[i]["out"]) for i in range(N_CORES)]
    full = np.concatenate(shards, axis=0).reshape(4, 2048, EMB)
    return full.astype(np.float32, copy=False), res


def kernel(**inputs) -> np.ndarray:
    return run(inputs)[0]
